# revision 3
# baseline (speedup 1.0000x reference)
"""Cross-attention (S2Audio) Trainium2 Bass kernel.

Sharding: data-parallel over the clip batch B=8 -> one batch element per
NeuronCore.  Per core the kernel computes, for its batch element b:

  q = (audio_patch + pos_a) @ q_w.T + q_b          (1568, 768)
  k,v = (s_x_patch + pos_s) @ kv_w.T + kv_b        (1568, 768) each
  out = softmax(q k^T / sqrt(64)) v  per 12 heads  -> proj -> (1568, 768)

Host prep is layout/elementwise only: weight transposes, positional-embedding
combine + add (O(N*D)), bf16 casts, sharding slices.  All matmuls/softmax run
on device.

On-device layout/dtype strategy:
  * matmul operands are bf16 (PE runs fp32 matmuls at 4 cycles/row vs 1 for
    bf16); every accumulation is fp32 in PSUM, softmax statistics fp32.
  * activations arrive feature-major (host-transposed) as x_feat [768, tok].
  * K projection produces feature-major k_feat [768, 1568] (lhsT = W^T chunk,
    rhs = x_feat) so heads live on partitions (contraction dim of the scores
    matmul).  Q is produced the same way per 512-token block, just in time
    inside the attention loop.
  * V projection produces token-major v [1568, 768] (lhsT = x_feat chunk,
    rhs = W^T), stored interleaved [128, 12, 65] with a ones-column per head.
  * scores are computed TRANSPOSED: sT[nk, nq] = k_feat_h(chunk)^T @ q_feat_h,
    exp() applied on ScalarE straight out of PSUM with the 1/sqrt(64) scale
    fused, output bf16.  No max-subtraction (scores are O(+-6); exp is safe in
    fp32 and matches the reference softmax mathematically).
  * PV: out_aug[65, nq] = v_aug^T @ exp_sT accumulated over nk chunks; row 64
    (from the ones column) is the softmax denominator.  Normalization:
    DVE reciprocal (fp32) + K=1 fp32 broadcast matmul + DVE multiply.
  * O-projection back to token-major fp32, then DMA out.
"""

import numpy as np
from contextlib import ExitStack

B, T, NPATCH, APATCH, D, H = 8, 8, 196, 196, 768, 12
HD = D // H                      # 64
SCALE = float(HD) ** -0.5        # 0.125
NT = NPATCH * T                  # 1568 tokens (same count for q and kv side)
P = 128
DC = D // P                      # 6 feature chunks
N_CORES = 8

# token chunks (partition-dim tiling): 12 x 128 + 1 x 32
TOK_CHUNKS = [(i * P, min(P, NT - i * P)) for i in range((NT + P - 1) // P)]
# nq blocks for the attention/output stage
NQB = 512
NQ_BLOCKS = [(s, min(NQB, NT - s)) for s in range(0, NT, NQB)]

_CACHE: dict = {}
LAST: dict = {"exec_time_ns": None, "trace": None}


def _build_nc(qb_nz: bool, kb_nz: bool, vb_nz: bool, pb_nz: bool):
    import concourse.mybir as mybir
    from concourse import bacc
    from concourse.tile import TileContext

    f32 = mybir.dt.float32
    bf16 = mybir.dt.bfloat16
    AF = mybir.ActivationFunctionType

    nc = bacc.Bacc("TRN2", target_bir_lowering=False, debug=False,
                   num_devices=N_CORES)

    xsT = nc.dram_tensor("xsT", [D, NT], bf16, kind="ExternalInput")
    xaT = nc.dram_tensor("xaT", [D, NT], bf16, kind="ExternalInput")
    qwT = nc.dram_tensor("qwT", [D, D], bf16, kind="ExternalInput")
    kvwT = nc.dram_tensor("kvwT", [D, 2 * D], bf16, kind="ExternalInput")
    projT = nc.dram_tensor("projT", [D, D], bf16, kind="ExternalInput")
    qb = nc.dram_tensor("qb", [P, DC], f32, kind="ExternalInput") if qb_nz else None
    kb = nc.dram_tensor("kb", [P, DC], f32, kind="ExternalInput") if kb_nz else None
    vb = nc.dram_tensor("vb", [1, D], bf16, kind="ExternalInput") if vb_nz else None
    pb = nc.dram_tensor("pb", [1, D], bf16, kind="ExternalInput") if pb_nz else None
    out = nc.dram_tensor("out", [NT, D], f32, kind="ExternalOutput")

    with TileContext(nc) as tc, ExitStack() as ctx:
        consts = ctx.enter_context(tc.tile_pool(name="consts", bufs=1))
        persist = ctx.enter_context(tc.tile_pool(name="persist", bufs=1))

        ones_bf = consts.tile([1, P], bf16, tag="ones_bf")
        nc.gpsimd.memset(ones_bf[:], 1.0)
        ones_f32 = consts.tile([1, HD], f32, tag="ones_f32")
        nc.gpsimd.memset(ones_f32[:], 1.0)
        qb_sb = kb_sb = vb_sb = pb_sb = None
        if qb_nz:
            qb_sb = consts.tile([P, DC], f32, tag="qb")
            nc.sync.dma_start(qb_sb[:], qb[:])
        if kb_nz:
            kb_sb = consts.tile([P, DC], f32, tag="kb")
            nc.sync.dma_start(kb_sb[:], kb[:])
        if vb_nz:
            vb_sb = consts.tile([1, D], bf16, tag="vb")
            nc.sync.dma_start(vb_sb[:], vb[:])
        if pb_nz:
            pb_sb = consts.tile([1, D], bf16, tag="pb")
            nc.sync.dma_start(pb_sb[:], pb[:])

        # persistent SBUF tensors: K (feature-major) and V (token-major)
        k_feat = [persist.tile([P, NT], bf16, tag=f"k_feat{c}", name=f"k_feat{c}")
                  for c in range(DC)]
        v_st = [persist.tile([P, H, HD + 1], bf16, tag=f"v{i}", name=f"v{i}")
                for i in range(len(TOK_CHUNKS))]

        # ---------------- phase 1: K and V projections ----------------
        with ExitStack() as ph:
            wtp = ph.enter_context(tc.tile_pool(name="wtp", bufs=1))
            xfp = ph.enter_context(tc.tile_pool(name="xfp", bufs=1))
            ps1 = ph.enter_context(tc.tile_pool(name="ps1", bufs=6, space="PSUM"))

            kvw_sb = wtp.tile([P, DC, 2 * D], bf16, tag="kvw", name="kvw")
            nc.sync.dma_start(kvw_sb[:], kvwT.rearrange("(c p) d -> p c d", p=P))

            xs_feat = [xfp.tile([P, NT], bf16, tag=f"xsf{c}", name=f"xsf{c}")
                       for c in range(DC)]
            for c in range(DC):
                nc.sync.dma_start(xs_feat[c][:], xsT[c * P:(c + 1) * P, :])

            # K projection (feature-major)
            for m in range(DC):
                for (n0, nw) in NQ_BLOCKS:
                    ps = ps1.tile([P, NQB], f32, tag="big", name="kproj")
                    for c in range(DC):
                        nc.tensor.matmul(ps[:, :nw],
                                         kvw_sb[:, c, m * P:(m + 1) * P],
                                         xs_feat[c][:, n0:n0 + nw],
                                         start=(c == 0), stop=(c == DC - 1))
                    if kb_nz:
                        nc.scalar.activation(k_feat[m][:, n0:n0 + nw],
                                             ps[:, :nw], AF.Identity,
                                             bias=kb_sb[:, m:m + 1])
                    else:
                        nc.vector.tensor_copy(k_feat[m][:, n0:n0 + nw],
                                              ps[:, :nw])

            # V projection (token-major, interleaved with ones column)
            for ti, (t0, tw) in enumerate(TOK_CHUNKS):
                for half in range(2):
                    ps = ps1.tile([P, NQB], f32, tag="big", name="vproj")
                    for c in range(DC):
                        nc.tensor.matmul(
                            ps[:tw, :384],
                            xs_feat[c][:, t0:t0 + tw],
                            kvw_sb[:, c, D + half * 384:D + (half + 1) * 384],
                            start=(c == 0), stop=(c == DC - 1 and not vb_nz))
                    if vb_nz:
                        nc.tensor.matmul(
                            ps[:tw, :384], ones_bf[:, :tw],
                            vb_sb[:, half * 384:(half + 1) * 384],
                            start=False, stop=True)
                    nc.vector.tensor_copy(
                        v_st[ti][:tw, half * 6:(half + 1) * 6, :HD],
                        ps[:tw, :384].rearrange("p (h d) -> p h d", d=HD))
                nc.vector.memset(v_st[ti][:tw, :, HD:], 1.0)

        # -------- phase 2: per-block Q proj + attention + O-proj --------
        with ExitStack() as ph:
            qwp = ph.enter_context(tc.tile_pool(name="qwp", bufs=1))
            pwp = ph.enter_context(tc.tile_pool(name="pwp", bufs=1))
            xfb = ph.enter_context(tc.tile_pool(name="xfb", bufs=2))
            qfb = ph.enter_context(tc.tile_pool(name="qfb", bufs=2))
            expp = ph.enter_context(tc.tile_pool(name="expp", bufs=15))
            ofp = ph.enter_context(tc.tile_pool(name="ofp", bufs=2))
            otp = ph.enter_context(tc.tile_pool(name="otp", bufs=2))
            nrm = ph.enter_context(tc.tile_pool(name="nrm", bufs=3))
            ps2 = ph.enter_context(tc.tile_pool(name="ps2", bufs=4, space="PSUM"))
            pvps = ph.enter_context(tc.tile_pool(name="pvps", bufs=2, space="PSUM"))
            bcps = ph.enter_context(tc.tile_pool(name="bcps", bufs=2, space="PSUM"))

            qw_sb = qwp.tile([P, DC, D], bf16, tag="qw", name="qw")
            nc.sync.dma_start(qw_sb[:], qwT.rearrange("(c p) d -> p c d", p=P))
            pw_sb = pwp.tile([P, DC, D], bf16, tag="pw", name="pw")
            nc.sync.dma_start(pw_sb[:], projT.rearrange("(c p) d -> p c d", p=P))

            for (n0, nw) in NQ_BLOCKS:
                # Q for this block: load (already feature-major) + project
                xa_feat = [xfb.tile([P, NQB], bf16, tag=f"xaf{c}", name=f"xaf{c}")
                           for c in range(DC)]
                for c in range(DC):
                    nc.sync.dma_start(xa_feat[c][:, :nw],
                                      xaT[c * P:(c + 1) * P, n0:n0 + nw])
                q_feat = [qfb.tile([P, NQB], bf16, tag=f"qf{c}", name=f"qf{c}")
                          for c in range(DC)]
                for m in range(DC):
                    ps = ps2.tile([P, NQB], f32, tag="big", name="qproj")
                    for c in range(DC):
                        nc.tensor.matmul(ps[:, :nw],
                                         qw_sb[:, c, m * P:(m + 1) * P],
                                         xa_feat[c][:, :nw],
                                         start=(c == 0), stop=(c == DC - 1))
                    if qb_nz:
                        nc.scalar.activation(q_feat[m][:, :nw], ps[:, :nw],
                                             AF.Identity, bias=qb_sb[:, m:m + 1])
                    else:
                        nc.vector.tensor_copy(q_feat[m][:, :nw], ps[:, :nw])

                out_feat = [ofp.tile([P, NQB], bf16, tag=f"of{c}", name=f"of{c}")
                            for c in range(DC)]
                for h in range(H):
                    hc, hp = h // 2, (h % 2) * HD
                    exp_tiles = []
                    # scores^T chunks + fused scale+exp (bf16 out)
                    for ti, (t0, tw) in enumerate(TOK_CHUNKS):
                        ps = ps2.tile([P, NQB], f32, tag="big", name="score")
                        nc.tensor.matmul(
                            ps[:tw, :nw],
                            k_feat[hc][hp:hp + HD, t0:t0 + tw],
                            q_feat[hc][hp:hp + HD, :nw],
                            start=True, stop=True)
                        et = expp.tile([P, NQB], bf16, tag="exp", name="exp")
                        nc.scalar.activation(et[:tw, :nw], ps[:tw, :nw],
                                             AF.Exp, scale=SCALE)
                        exp_tiles.append(et)
                    # PV with ones-row -> [65, nw]; row 64 = denominator
                    pv = pvps.tile([HD + 1, NQB], f32, tag="pv", name="pv")
                    for ti, (t0, tw) in enumerate(TOK_CHUNKS):
                        nc.tensor.matmul(pv[:, :nw],
                                         v_st[ti][:tw, h, :],
                                         exp_tiles[ti][:tw, :nw],
                                         start=(ti == 0),
                                         stop=(ti == len(TOK_CHUNKS) - 1))
                    # normalize: fp32 reciprocal of denom, broadcast over
                    # partitions via a K=1 fp32 matmul, then one DVE mul
                    rec = nrm.tile([1, NQB], f32, tag="rec", name="rec")
                    nc.vector.reciprocal(rec[:, :nw], pv[HD:HD + 1, :nw])
                    bc = bcps.tile([HD, NQB], f32, tag="bc", name="bc")
                    nc.tensor.matmul(bc[:, :nw], ones_f32[:1, :HD],
                                     rec[:, :nw], start=True, stop=True)
                    bcs = nrm.tile([HD, NQB], f32, tag="bcs", name="bcs")
                    nc.vector.tensor_copy(bcs[:, :nw], bc[:, :nw])
                    nc.vector.tensor_mul(out_feat[hc][hp:hp + HD, :nw],
                                         pv[:HD, :nw], bcs[:, :nw])

                # O-projection for this block (token-major out)
                for (c0, cw) in [(c, min(P, nw - c)) for c in range(0, nw, P)]:
                    ot = otp.tile([P, D], f32, tag="ot", name="ot")
                    for half in range(2):
                        ps = ps2.tile([P, NQB], f32, tag="big", name="oproj")
                        for c in range(DC):
                            nc.tensor.matmul(
                                ps[:cw, :384],
                                out_feat[c][:, c0:c0 + cw],
                                pw_sb[:, c, half * 384:(half + 1) * 384],
                                start=(c == 0), stop=(c == DC - 1 and not pb_nz))
                        if pb_nz:
                            nc.tensor.matmul(
                                ps[:cw, :384], ones_bf[:, :cw],
                                pb_sb[:, half * 384:(half + 1) * 384],
                                start=False, stop=True)
                        nc.vector.tensor_copy(
                            ot[:cw, half * 384:(half + 1) * 384], ps[:cw, :384])
                    nc.sync.dma_start(out[n0 + c0:n0 + c0 + cw, :], ot[:cw, :])

    nc.finalize()
    return nc


def kernel(**inputs) -> np.ndarray:
    import ml_dtypes
    bf = ml_dtypes.bfloat16

    s_x = np.asarray(inputs["s_x"], np.float32)
    audio = np.asarray(inputs["audio"], np.float32)
    q_w = np.asarray(inputs["q_w"], np.float32)
    q_b = np.asarray(inputs["q_b"], np.float32)
    kv_w = np.asarray(inputs["kv_w"], np.float32)
    kv_b = np.asarray(inputs["kv_b"], np.float32)
    proj_w = np.asarray(inputs["proj_w"], np.float32)
    proj_b = np.asarray(inputs["proj_b"], np.float32)

    # host prep: layout + O(N*D) positional add + bf16 casts only
    pos_s = (np.asarray(inputs["clip_space_pos"], np.float32)[:, None, :]
             + np.asarray(inputs["clip_temporal_pos"], np.float32)[None, :, :]
             ).reshape(NT, D)
    pos_a = (np.asarray(inputs["audio_space_pos"], np.float32)[:, None, :]
             + np.asarray(inputs["audio_temporal_pos"], np.float32)[None, :, :]
             ).reshape(NT, D)
    qwT = np.ascontiguousarray(q_w.T).astype(bf)
    kvwT = np.ascontiguousarray(kv_w.T).astype(bf)
    projT = np.ascontiguousarray(proj_w.T).astype(bf)
    qb_nz = bool(np.any(q_b))
    kb_nz = bool(np.any(kv_b[:D]))
    vb_nz = bool(np.any(kv_b[D:]))
    pb_nz = bool(np.any(proj_b))

    key = (qb_nz, kb_nz, vb_nz, pb_nz)
    if key not in _CACHE:
        _CACHE[key] = _build_nc(*key)
    nc = _CACHE[key]

    shared = {"qwT": qwT, "kvwT": kvwT, "projT": projT}
    if qb_nz:
        shared["qb"] = np.ascontiguousarray(q_b.reshape(DC, P).T)
    if kb_nz:
        shared["kb"] = np.ascontiguousarray(kv_b[:D].reshape(DC, P).T)
    if vb_nz:
        shared["vb"] = np.ascontiguousarray(kv_b[D:].reshape(1, D)).astype(bf)
    if pb_nz:
        shared["pb"] = np.ascontiguousarray(proj_b.reshape(1, D)).astype(bf)

    in_maps = []
    for b in range(N_CORES):
        m = dict(shared)
        m["xsT"] = np.ascontiguousarray(
            (s_x[1:, b * T:(b + 1) * T, :].reshape(NT, D) + pos_s).T).astype(bf)
        m["xaT"] = np.ascontiguousarray(
            (audio[2:, b * T:(b + 1) * T, :].reshape(NT, D) + pos_a).T).astype(bf)
        in_maps.append(m)

    from concourse.bass_utils import run_bass_kernel_spmd
    res = run_bass_kernel_spmd(nc, in_maps, core_ids=list(range(N_CORES)))
    LAST["exec_time_ns"] = res.exec_time_ns
    LAST["trace"] = res.instructions_and_trace

    out_full = np.empty((2 + APATCH, B * T, D), np.float32)
    out_full[:2] = audio[:2]
    for b in range(N_CORES):
        out_full[2:, b * T:(b + 1) * T, :] = \
            res.results[b]["out"].reshape(APATCH, T, D)
    return out_full



# revision 18
# speedup vs baseline: 1.9766x; 1.9766x over previous
"""Cross-attention (S2Audio) Trainium2 Bass kernel.

Sharding: data-parallel over the clip batch B=8 -> one batch element per
NeuronCore.  Per core the kernel computes, for its batch element b:

  q = (audio_patch + pos_a) @ q_w.T + q_b          (1568, 768)
  k,v = (s_x_patch + pos_s) @ kv_w.T + kv_b        (1568, 768) each
  out = softmax(q k^T / sqrt(64)) v  per 12 heads  -> proj -> (1568, 768)

Host prep is layout/elementwise only: weight transposes, positional-embedding
combine + add (O(N*D)), bf16 casts, sharding slices.  All matmuls/softmax run
on device.

Performance-critical structure (v2):
  * The TRN2 PE clock-gates to 1.2 GHz (HAM K=4/8) whenever it idles; dense
    back-to-back matmul emission keeps it at 2.4 GHz.  All per-head serial
    work (softmax normalization) is OFF the PE queue: denominators come free
    from a ones-column in the PV matmul, reciprocals are batched per block on
    DVE ([12, nq] in one instruction), the partition-broadcast runs on the
    otherwise-idle GpSimd engine, and the final scale is an in-place DVE mul.
    The whole normalize + O-projection of block b-1 is software-pipelined
    into block b's head loop.
  * Scores matmuls have K=64 (head dim) -> 64x128 PE row tiling: the two
    heads of a pair live on SBUF partitions 0-63 / 64-127, their score
    matmuls are emitted interleaved (tile_position (0,0)/(64,0)) so they
    stream CONCURRENTLY through the two 64-row halves of the PE array.
  * Both heads' scores for a token chunk land in one 2-bank PSUM quad tile;
    a single ScalarE ACTIVATE [tw, 2*nq] applies exp to the pair (fused
    1/sqrt(64) scale, bf16 out) - ScalarE instruction count matters because
    exp is the attention-phase throughput floor.
  * PV of pair c-1 is emitted BEFORE scores of pair c so ready PE work never
    queues behind score matmuls that are gated on the exp pipeline.
  * Weight/activation DMAs are issued per-chunk, compute-first order, so the
    first K-proj matmul starts ~4us in and phase transitions have no PE gap.
"""

import numpy as np
from contextlib import ExitStack

B, T, NPATCH, APATCH, D, H = 8, 8, 196, 196, 768, 12
HD = D // H                      # 64
SCALE = float(HD) ** -0.5        # 0.125
NT = NPATCH * T                  # 1568 tokens (same count for q and kv side)
P = 128
DC = D // P                      # 6 feature chunks
N_CORES = 8

# token chunks (partition-dim tiling): 12 x 128 + 1 x 32
TOK_CHUNKS = [(i * P, min(P, NT - i * P)) for i in range((NT + P - 1) // P)]
NTC = len(TOK_CHUNKS)            # 13
# nq blocks for the attention/output stage
NQB = 512
NQ_BLOCKS = [(s, min(NQB, NT - s)) for s in range(0, NT, NQB)]
NPAIR = H // 2                   # 6 head pairs

_CACHE: dict = {}
LAST: dict = {"exec_time_ns": None, "trace": None}


def _build_nc(qb_nz: bool, kb_nz: bool, vb_nz: bool, pb_nz: bool):
    import concourse.mybir as mybir
    from concourse import bacc
    from concourse.tile import TileContext

    f32 = mybir.dt.float32
    bf16 = mybir.dt.bfloat16
    AF = mybir.ActivationFunctionType

    nc = bacc.Bacc("TRN2", target_bir_lowering=False, debug=False,
                   num_devices=N_CORES)

    xsT = nc.dram_tensor("xsT", [D, NT], bf16, kind="ExternalInput")
    xaT = nc.dram_tensor("xaT", [D, NT], bf16, kind="ExternalInput")
    qwT = nc.dram_tensor("qwT", [D, D], bf16, kind="ExternalInput")
    kvwT = nc.dram_tensor("kvwT", [D, 2 * D], bf16, kind="ExternalInput")
    projT = nc.dram_tensor("projT", [D, D], bf16, kind="ExternalInput")
    qb = nc.dram_tensor("qb", [P, DC], f32, kind="ExternalInput") if qb_nz else None
    kb = nc.dram_tensor("kb", [P, DC], f32, kind="ExternalInput") if kb_nz else None
    vb = nc.dram_tensor("vb", [1, D], bf16, kind="ExternalInput") if vb_nz else None
    pb = nc.dram_tensor("pb", [1, D], bf16, kind="ExternalInput") if pb_nz else None
    out = nc.dram_tensor("out", [NT, D], f32, kind="ExternalOutput")

    with TileContext(nc) as tc, ExitStack() as ctx:
        consts = ctx.enter_context(tc.tile_pool(name="consts", bufs=1))
        persist = ctx.enter_context(tc.tile_pool(name="persist", bufs=1))

        ones_bf = consts.tile([1, P], bf16, tag="ones_bf")
        nc.gpsimd.memset(ones_bf[:], 1.0)
        qb_sb = kb_sb = vb_sb = pb_sb = None
        if qb_nz:
            qb_sb = consts.tile([P, DC], f32, tag="qb")
            nc.sync.dma_start(qb_sb[:], qb[:])
        if kb_nz:
            kb_sb = consts.tile([P, DC], f32, tag="kb")
            nc.sync.dma_start(kb_sb[:], kb[:])
        if vb_nz:
            vb_sb = consts.tile([1, D], bf16, tag="vb")
            nc.sync.dma_start(vb_sb[:], vb[:])
        if pb_nz:
            pb_sb = consts.tile([1, D], bf16, tag="pb")
            nc.sync.dma_start(pb_sb[:], pb[:])

        # persistent SBUF tensors: K (feature-major) and V (token-major)
        k_feat = [persist.tile([P, NT], bf16, tag=f"k_feat{c}", name=f"k_feat{c}")
                  for c in range(DC)]
        v_st = [persist.tile([P, H, HD + 1], bf16, tag=f"v{i}", name=f"v{i}")
                for i in range(NTC)]

        # phase-2 weights, prefetched during phase 1
        qw_sb = persist.tile([P, DC, D], bf16, tag="qw", name="qw")
        pw_sb = persist.tile([P, DC, D], bf16, tag="pw", name="pw")

        # ---------------- phase 1: K and V projections ----------------
        with ExitStack() as ph:
            wtp = ph.enter_context(tc.tile_pool(name="wtp", bufs=1))
            xfp = ph.enter_context(tc.tile_pool(name="xfp", bufs=1))
            ps1 = ph.enter_context(tc.tile_pool(name="ps1", bufs=6, space="PSUM"))

            kvw_sb = wtp.tile([P, DC, 2 * D], bf16, tag="kvw", name="kvw")
            xs_feat = [xfp.tile([P, NT], bf16, tag=f"xsf{c}", name=f"xsf{c}")
                       for c in range(DC)]
            # compute-first DMA order: K-proj can start after the first
            # kvw/xs chunk pair lands; phase-2 weights stream in behind.
            for c in range(DC):
                nc.sync.dma_start(kvw_sb[:, c, :], kvwT[c * P:(c + 1) * P, :])
                nc.sync.dma_start(xs_feat[c][:], xsT[c * P:(c + 1) * P, :])
            nc.sync.dma_start(qw_sb[:], qwT.rearrange("(c p) d -> p c d", p=P))
            nc.sync.dma_start(pw_sb[:], projT.rearrange("(c p) d -> p c d", p=P))

            # K projection (feature-major)
            for m in range(DC):
                for (n0, nw) in NQ_BLOCKS:
                    ps = ps1.tile([P, NQB], f32, tag="big", name="kproj")
                    for c in range(DC):
                        nc.tensor.matmul(ps[:, :nw],
                                         kvw_sb[:, c, m * P:(m + 1) * P],
                                         xs_feat[c][:, n0:n0 + nw],
                                         start=(c == 0), stop=(c == DC - 1))
                    if kb_nz:
                        nc.scalar.activation(k_feat[m][:, n0:n0 + nw],
                                             ps[:, :nw], AF.Identity,
                                             bias=kb_sb[:, m:m + 1])
                    else:
                        nc.vector.tensor_copy(k_feat[m][:, n0:n0 + nw],
                                              ps[:, :nw])

            # V projection (token-major, interleaved with ones column)
            for ti, (t0, tw) in enumerate(TOK_CHUNKS):
                for half in range(2):
                    ps = ps1.tile([P, NQB], f32, tag="big", name="vproj")
                    for c in range(DC):
                        nc.tensor.matmul(
                            ps[:tw, :384],
                            xs_feat[c][:, t0:t0 + tw],
                            kvw_sb[:, c, D + half * 384:D + (half + 1) * 384],
                            start=(c == 0), stop=(c == DC - 1 and not vb_nz))
                    if vb_nz:
                        nc.tensor.matmul(
                            ps[:tw, :384], ones_bf[:, :tw],
                            vb_sb[:, half * 384:(half + 1) * 384],
                            start=False, stop=True)
                    nc.vector.tensor_copy(
                        v_st[ti][:tw, half * 6:(half + 1) * 6, :HD],
                        ps[:tw, :384].rearrange("p (h d) -> p h d", d=HD))
                nc.vector.memset(v_st[ti][:tw, :, HD:], 1.0)

        # -------- phase 2: per-block Q proj + attention + O-proj --------
        with ExitStack() as ph:
            xfb = ph.enter_context(tc.tile_pool(name="xfb", bufs=2))
            qfb = ph.enter_context(tc.tile_pool(name="qfb", bufs=2))
            expp = ph.enter_context(tc.tile_pool(name="expp", bufs=4))
            ofp = ph.enter_context(tc.tile_pool(name="ofp", bufs=2))
            otp = ph.enter_context(tc.tile_pool(name="otp", bufs=2))
            nrm = ph.enter_context(tc.tile_pool(name="nrm", bufs=1))
            bcp = ph.enter_context(tc.tile_pool(name="bcp", bufs=3))
            scq = ph.enter_context(tc.tile_pool(name="scq", bufs=2, space="PSUM"))
            pvps = ph.enter_context(tc.tile_pool(name="pvps", bufs=2, space="PSUM"))
            prj = ph.enter_context(tc.tile_pool(name="prj", bufs=2, space="PSUM"))

            def emit_attn_pair(c, blk, nw):
                """Attention for head pair (2c, 2c+1).  The two heads'
                score matmuls go to PE row tiles (0,0)/(64,0) back-to-back
                so they stream concurrently; one ScalarE ACT applies exp to
                the 2-bank quad; PV matmuls trail scores by one token chunk
                so PE and ScalarE both stay continuously fed.  Row 64 of
                each pv (from the V ones-column) is the softmax
                denominator."""
                q_feat = blk["q_feat"]
                pvt = [pvps.tile([HD + 1, NQB], f32, tag="pv", name="pv")
                       for _ in range(2)]

                def pv_chunk(ti, et):
                    t0, tw = TOK_CHUNKS[ti]
                    for par in range(2):
                        nc.tensor.matmul(pvt[par][:, :nw],
                                         v_st[ti][:tw, 2 * c + par, :],
                                         et[:tw, par, :nw],
                                         start=(ti == 0),
                                         stop=(ti == NTC - 1))

                prev_et = None
                for ti, (t0, tw) in enumerate(TOK_CHUNKS):
                    qd = scq.tile([P, 2, NQB], f32, tag="quad", name="squad")
                    for par in range(2):
                        hp = par * HD
                        nc.tensor.matmul(
                            qd[:tw, par, :nw],
                            k_feat[c][hp:hp + HD, t0:t0 + tw],
                            q_feat[c][hp:hp + HD, :nw],
                            start=True, stop=True)
                    et = expp.tile([P, 2, NQB], bf16, tag="exp", name="exp")
                    nc.scalar.activation(et[:tw, :, :nw], qd[:tw, :, :nw],
                                         AF.Exp, scale=SCALE)
                    if prev_et is not None:
                        pv_chunk(ti - 1, prev_et)
                    prev_et = et
                pv_chunk(NTC - 1, prev_et)

                # drain: denominators free-major onto partition 0 (DVE APs
                # need 32-aligned partition bases, so a [12, nq] gather is
                # staged via DMA in norm_stage 0), numerators into out_feat
                # (pre-normalization; scaled in-place next block)
                for par in range(2):
                    h = 2 * c + par
                    nc.vector.tensor_copy(blk["den_st"][0:1, h, :nw],
                                          pvt[par][HD:HD + 1, :nw])
                    nc.vector.tensor_copy(
                        blk["out_feat"][c][par * HD:(par + 1) * HD, :nw],
                        pvt[par][:HD, :nw])

            def norm_stage(blk, stage):
                """Normalize + O-proj of a prior block, split into 6 stages
                interleaved into the successor block's head loop."""
                if blk is None:
                    return
                nw, n0 = blk["nw"], blk["n0"]
                if stage in (0, 1):
                    # partition_broadcast requires dst base partition 0 and
                    # tensor_tensor requires equal input bases: broadcast the
                    # odd head's reciprocal to all 128 partitions, overwrite
                    # partitions 0-63 with the even head's (gpsimd FIFO
                    # orders the writes), then one full-pair mul at base 0.
                    for c in range(3 * stage, 3 * (stage + 1)):
                        bc = bcp.tile([P, NQB], f32, tag="bc", name="bc")
                        nc.gpsimd.partition_broadcast(
                            bc[:, :nw], blk["rec_st"][0:1, 2 * c + 1, :nw])
                        nc.gpsimd.partition_broadcast(
                            bc[:HD, :nw], blk["rec_st"][0:1, 2 * c, :nw])
                        nc.vector.tensor_mul(blk["out_feat"][c][:, :nw],
                                             blk["out_feat"][c][:, :nw],
                                             bc[:, :nw])
                elif stage in (2, 3):
                    # O-projection chunk groups + output DMA
                    chunks = [(cc, min(P, nw - cc)) for cc in range(0, nw, P)]
                    lo = (stage - 2) * 2
                    for (c0, cw) in chunks[lo:lo + 2]:
                        ot = otp.tile([P, D], f32, tag="ot", name="ot")
                        for half in range(2):
                            ps = prj.tile([P, NQB], f32, tag="prj", name="oproj")
                            for c in range(DC):
                                nc.tensor.matmul(
                                    ps[:cw, :384],
                                    blk["out_feat"][c][:, c0:c0 + cw],
                                    pw_sb[:, c, half * 384:(half + 1) * 384],
                                    start=(c == 0),
                                    stop=(c == DC - 1 and not pb_nz))
                            if pb_nz:
                                nc.tensor.matmul(
                                    ps[:cw, :384], ones_bf[:, :cw],
                                    pb_sb[:, half * 384:(half + 1) * 384],
                                    start=False, stop=True)
                            nc.vector.tensor_copy(
                                ot[:cw, half * 384:(half + 1) * 384],
                                ps[:cw, :384])
                        nc.sync.dma_start(out[n0 + c0:n0 + c0 + cw, :],
                                          ot[:cw, :])

            prev = None
            for bi, (n0, nw) in enumerate(NQ_BLOCKS):
                blk = {"n0": n0, "nw": nw}
                # load + project Q for this block (feature-major)
                xa_feat = [xfb.tile([P, NQB], bf16, tag=f"xaf{c}",
                                    name=f"xaf{c}") for c in range(DC)]
                for c in range(DC):
                    nc.sync.dma_start(xa_feat[c][:, :nw],
                                      xaT[c * P:(c + 1) * P, n0:n0 + nw])
                q_feat = [qfb.tile([P, NQB], bf16, tag=f"qf{c}",
                                   name=f"qf{c}") for c in range(DC)]
                for m in range(DC):
                    ps = prj.tile([P, NQB], f32, tag="prj", name="qproj")
                    for c in range(DC):
                        nc.tensor.matmul(ps[:, :nw],
                                         qw_sb[:, c, m * P:(m + 1) * P],
                                         xa_feat[c][:, :nw],
                                         start=(c == 0), stop=(c == DC - 1))
                    if qb_nz:
                        nc.scalar.activation(q_feat[m][:, :nw], ps[:, :nw],
                                             AF.Identity, bias=qb_sb[:, m:m + 1])
                    else:
                        nc.vector.tensor_copy(q_feat[m][:, :nw], ps[:, :nw])
                blk["q_feat"] = q_feat
                blk["out_feat"] = [ofp.tile([P, NQB], bf16, tag=f"of{c}",
                                            name=f"of{c}") for c in range(DC)]
                blk["den_st"] = nrm.tile([1, H, NQB], f32, tag="denst",
                                         name="den_st")
                blk["den12"] = nrm.tile([H, NQB], f32, tag="den", name="den12")
                blk["rec12"] = nrm.tile([H, NQB], f32, tag="rec", name="rec12")
                blk["rec_st"] = nrm.tile([1, H, NQB], f32, tag="recst",
                                         name="rec_st")

                for c in range(NPAIR):
                    emit_attn_pair(c, blk, nw)
                    norm_stage(prev, c)
                # gather the 12 denominators to [12, nq] partitions via DMA
                # (engine APs need 32-aligned partition bases; DMA does not),
                # one batched DVE reciprocal, then scatter back free-major
                # for the gpsimd partition_broadcast reads next block.
                nc.sync.dma_start(blk["den12"][:, :nw],
                                  blk["den_st"][0:1, :, :nw])
                nc.vector.reciprocal(blk["rec12"][:, :nw],
                                     blk["den12"][:, :nw])
                nc.sync.dma_start(blk["rec_st"][0:1, :, :nw],
                                  blk["rec12"][:, :nw])
                prev = blk

            for stage in range(6):
                norm_stage(prev, stage)

    nc.finalize()
    return nc


def kernel(**inputs) -> np.ndarray:
    import ml_dtypes
    bf = ml_dtypes.bfloat16

    s_x = np.asarray(inputs["s_x"], np.float32)
    audio = np.asarray(inputs["audio"], np.float32)
    q_w = np.asarray(inputs["q_w"], np.float32)
    q_b = np.asarray(inputs["q_b"], np.float32)
    kv_w = np.asarray(inputs["kv_w"], np.float32)
    kv_b = np.asarray(inputs["kv_b"], np.float32)
    proj_w = np.asarray(inputs["proj_w"], np.float32)
    proj_b = np.asarray(inputs["proj_b"], np.float32)

    # host prep: layout + O(N*D) positional add + bf16 casts only
    pos_s = (np.asarray(inputs["clip_space_pos"], np.float32)[:, None, :]
             + np.asarray(inputs["clip_temporal_pos"], np.float32)[None, :, :]
             ).reshape(NT, D)
    pos_a = (np.asarray(inputs["audio_space_pos"], np.float32)[:, None, :]
             + np.asarray(inputs["audio_temporal_pos"], np.float32)[None, :, :]
             ).reshape(NT, D)
    qwT = np.ascontiguousarray(q_w.T).astype(bf)
    kvwT = np.ascontiguousarray(kv_w.T).astype(bf)
    projT = np.ascontiguousarray(proj_w.T).astype(bf)
    qb_nz = bool(np.any(q_b))
    kb_nz = bool(np.any(kv_b[:D]))
    vb_nz = bool(np.any(kv_b[D:]))
    pb_nz = bool(np.any(proj_b))

    key = (qb_nz, kb_nz, vb_nz, pb_nz)
    if key not in _CACHE:
        _CACHE[key] = _build_nc(*key)
    nc = _CACHE[key]

    shared = {"qwT": qwT, "kvwT": kvwT, "projT": projT}
    if qb_nz:
        shared["qb"] = np.ascontiguousarray(q_b.reshape(DC, P).T)
    if kb_nz:
        shared["kb"] = np.ascontiguousarray(kv_b[:D].reshape(DC, P).T)
    if vb_nz:
        shared["vb"] = np.ascontiguousarray(kv_b[D:].reshape(1, D)).astype(bf)
    if pb_nz:
        shared["pb"] = np.ascontiguousarray(proj_b.reshape(1, D)).astype(bf)

    in_maps = []
    for b in range(N_CORES):
        m = dict(shared)
        m["xsT"] = np.ascontiguousarray(
            (s_x[1:, b * T:(b + 1) * T, :].reshape(NT, D) + pos_s).T).astype(bf)
        m["xaT"] = np.ascontiguousarray(
            (audio[2:, b * T:(b + 1) * T, :].reshape(NT, D) + pos_a).T).astype(bf)
        in_maps.append(m)

    from concourse.bass_utils import run_bass_kernel_spmd
    res = run_bass_kernel_spmd(nc, in_maps, core_ids=list(range(N_CORES)))
    LAST["exec_time_ns"] = res.exec_time_ns
    LAST["trace"] = res.instructions_and_trace

    out_full = np.empty((2 + APATCH, B * T, D), np.float32)
    out_full[:2] = audio[:2]
    for b in range(N_CORES):
        out_full[2:, b * T:(b + 1) * T, :] = \
            res.results[b]["out"].reshape(APATCH, T, D)
    return out_full
